# revision 18
# baseline (speedup 1.0000x reference)
"""CARAFE content-aware upsampling on 8 Trainium2 NeuronCores (v2).

Full inputs: features (8, 256, 64, 64) f32, masks (8, 25, 128, 128) f32.
Full output: (8, 256, 128, 128) f32.  Data-parallel: one batch per core.

Math per batch (kernel 5x5, group 1, scale 2, pad 2):
  out[c, 2h+a, 2j+b] = sum_{dy,dx} f[c, h+dy-2, j+dx-2] * masks[5dy+dx, 2h+a, 2j+b]

Device strategy (per input row h, per c-half): accumulate into PSUM via
bf16 matmuls whose contraction dim is (input-row-pair x w'-window):
  psum[c(128), (a,ow)] += lhsT[p=(w',par), c].T @ T[p, (a,ow)]
lhsT = features in interleaved layout fI[p = 2w+par, k*C+c] = f[2k+par, w, c],
so a row-pair restricted to a w'-window is a contiguous partition range.
T = host-prebuilt banded mask (Toeplitz) tiles.  The w' windows are limited
by the PE base-partition rule (base in {0,32,64}, base-32 spans <= 32):

  R0: w' [0,20)  base 0,  covers j [0,18)   (ow [0,36))
  R1: w' [16,32) base 32, covers j [18,30)  (ow [36,60))
  RP: w' [0,36)  base 0,  covers j [30,34)  (ow [60,68))
  R2: w' [32,64) base 64, covers j [34,64)  (ow [68,128))

Each (h, region) is a <=3-link PSUM chain over row pairs P_{m-1}, P_m,
P_{m+1} (m = h>>1); slots whose dy falls outside [0,5) carry zeros in T.
Matmul out APs are 3-dim strided so PSUM ends up row-major (a, ow) and a
single [128,256] copy per (h, half) moves it to SBUF as bf16.

Banded tiles carry ~4.9 MiB instead of 10 MiB (baseline), features load
once (2 MiB instead of 6), output is 8 MiB bf16: ~15 MiB HBM traffic/core.
"""

import sys

if "/opt/trn_rl_repo" not in sys.path:
    sys.path.append("/opt/trn_rl_repo")

from contextlib import ExitStack

import numpy as np
import ml_dtypes

import concourse.bass as bass
import concourse.bacc as bacc
import concourse.mybir as mybir
import concourse.tile as tile
from concourse.ap import AP
from concourse.bass_utils import run_bass_kernel_spmd

N = 8
C = 256
H = 64
W = 64
HB = 8                       # input rows per block
NBLK = H // HB
FI_T = 2048                  # free elems per fI sub-tile (8 row-pairs x C)
OS_F = 2 * HB * 2 * 2 * W    # 4096 outS cols per block (half, hl, a, ow)
OS_AL = OS_F + 64

# (wb, wn, jlo, jhi): w'-window [wb, wb+wn), output-col range j [jlo, jhi)
REGIONS = [
    (0, 20, 0, 18),
    (16, 16, 18, 30),
    (0, 36, 30, 34),
    (32, 32, 34, 64),
]


def _reg_geom(reg):
    wb, wn, jlo, jhi = reg
    rows = 2 * wn
    cw = 2 * (jhi - jlo)          # output cols in chunk
    cols = 3 * HB * 2 * (jhi - jlo) * 2   # (t, hl, a, jrel, b2)
    return wb, wn, jlo, jhi, rows, cw, cols


def _rap(tile_ap, off, dims):
    return AP(tile_ap.tensor, tile_ap.offset + off, dims)


def build_carafe(nc, out_dtype=mybir.dt.bfloat16, repeat=1):
    total_mask = sum(NBLK * _reg_geom(r)[4] * _reg_geom(r)[6] for r in REGIONS)
    feat = nc.declare_dram_parameter("features", (128, 4 * FI_T), mybir.dt.bfloat16,
                                     isOutput=False)
    tope = nc.declare_dram_parameter("masks", (total_mask,), mybir.dt.bfloat16,
                                     isOutput=False)
    out = nc.declare_dram_parameter("out", (C, 2 * H, 2 * W), out_dtype, isOutput=True)

    ctx = ExitStack()
    with ctx:
        tc = ctx.enter_context(tile.TileContext(nc))
        pool = ctx.enter_context(tc.tile_pool(name="main", bufs=1))
        ppool = ctx.enter_context(tc.tile_pool(name="psum", bufs=1, space="PSUM"))

        # features: fI0a holds pairs 0-4 (all block 0 needs), fI0b pairs 5-7,
        # then three 8-pair tiles.  Pair k lives in tile _fi_tile(k) at column
        # _fi_off(k).
        fI0a = pool.tile([128, 5 * C], mybir.dt.bfloat16, tag="fI0a", name="fI0a")
        fI0b = pool.tile([128, 3 * C], mybir.dt.bfloat16, tag="fI0b", name="fI0b")
        fIq = [pool.tile([128, FI_T], mybir.dt.bfloat16, tag=f"fI{q}", name=f"fI{q}")
               for q in (1, 2, 3)]

        def _fi(k):
            if k < 5:
                return fI0a, k * C, 5 * C
            if k < 8:
                return fI0b, (k - 5) * C, 3 * C
            return fIq[(k >> 3) - 1], (k & 7) * C, FI_T

        def load_fi(q):
            nc.sync.dma_start(
                _rap(fIq[q - 1][:, :], 0, [[FI_T, 128], [1, FI_T]]),
                _rap(feat[:, :], q * FI_T, [[4 * FI_T, 128], [1, FI_T]]))

        # banded mask tiles: ring of 2 per region; rows live at partitions
        # [2*wb, 2*wb+rows)
        tT = []
        for ri, reg in enumerate(REGIONS):
            wb, wn, jlo, jhi, rows, cw, cols = _reg_geom(reg)
            tT.append([pool.tile([2 * wb + rows, cols], mybir.dt.bfloat16,
                                 tag=f"t{ri}_{i}", name=f"t{ri}_{i}")
                       for i in range(2)])

        outS = [pool.tile([128, OS_AL], out_dtype, tag=f"outS_{i}", name=f"outS_{i}")
                for i in range(2)]
        psum = [ppool.tile([128, 512], mybir.dt.float32, tag=f"ps_{i}", name=f"ps_{i}")
                for i in range(8)]

        reg_base = []
        acc = 0
        for reg in REGIONS:
            reg_base.append(acc)
            acc += NBLK * _reg_geom(reg)[4] * _reg_geom(reg)[6]

        copy_engines = [nc.vector.tensor_copy, nc.scalar.copy]

        def load_masks(it, order=(0, 1, 2, 3)):
            blk = it % NBLK
            ring = it % 2
            for ri in order:
                reg = REGIONS[ri]
                wb, wn, jlo, jhi, rows, cw, cols = _reg_geom(reg)
                t = tT[ri][ring]
                eng = nc.scalar if ri % 2 == 0 else nc.sync
                eng.dma_start(
                    _rap(t[:, :], 2 * wb * cols, [[cols, rows], [1, cols]]),
                    _rap(tope[:], reg_base[ri] + blk * rows * cols,
                         [[cols, rows], [1, cols]]))

        niter = NBLK * repeat
        # startup order: fI0a and R0's tile gate the first matmuls -- issue
        # them at the head of their queues, fI0b behind the block-0 masks
        nc.sync.dma_start(
            _rap(fI0a[:, :], 0, [[5 * C, 128], [1, 5 * C]]),
            _rap(feat[:, :], 0, [[4 * FI_T, 128], [1, 5 * C]]))
        load_masks(0)
        nc.scalar.dma_start(
            _rap(fI0b[:, :], 0, [[3 * C, 128], [1, 3 * C]]),
            _rap(feat[:, :], 5 * C, [[4 * FI_T, 128], [1, 3 * C]]))
        for it in range(niter):
            blk = it % NBLK
            ring = it % 2
            # prefetch next block's banded tiles before this block's compute
            if it + 1 < niter:
                load_masks(it + 1)
            if it == 0:
                for q in (1, 2, 3):
                    load_fi(q)
            oS = outS[ring]

            def chain(hl, half, ri):
                h = HB * blk + hl
                m = h >> 1
                links = [t_ for t_ in range(3) if 0 <= m - 1 + t_ < 32]
                wb, wn, jlo, jhi, rows, cw, cols = _reg_geom(REGIONS[ri])
                t = tT[ri][ring]
                ps = psum[(2 * h + half) % 8]
                out_ap = _rap(ps[:, :], 2 * jlo, [[512, 128], [128, 2], [1, cw]])
                for i, tt in enumerate(links):
                    k = m - 1 + tt
                    ft, foff, fpitch = _fi(k)
                    lhs = _rap(ft[:, :],
                               2 * wb * fpitch + foff + half * 128,
                               [[fpitch, rows], [1, 128]])
                    rhs = _rap(t[:, :],
                               2 * wb * cols + tt * HB * 2 * cw + hl * 2 * cw,
                               [[cols, rows], [1, 2 * cw]])
                    nc.tensor.matmul(out_ap, lhs, rhs,
                                     start=(i == 0), stop=(i == len(links) - 1))

            def copy_out(hl, half):
                h = HB * blk + hl
                ps = psum[(2 * h + half) % 8]
                cp = copy_engines[(2 * h + half) % 2]
                cp(oS[:, half * 2048 + hl * 256:half * 2048 + (hl + 1) * 256],
                   ps[:, 0:256])

            for hl in range(HB):
                if it == 0 and hl % 4 == 0:
                    # block 0: region-major over the half-block so PE starts
                    # as soon as the first region tile lands
                    for ri in range(len(REGIONS)):
                        for hl2 in range(hl, hl + 4):
                            for half in (0, 1):
                                chain(hl2, half, ri)
                if it == 0:
                    for half in (0, 1):
                        copy_out(hl, half)
                else:
                    for half in (0, 1):
                        for ri in range(len(REGIONS)):
                            chain(hl, half, ri)
                        copy_out(hl, half)
                fine = it == niter - 1
                if (hl % 2 == 1) if fine else (hl in (3, 7)):
                    segw = 512 if fine else 1024
                    seg = hl // 2 if fine else hl // 4
                    dst = _rap(out[:, :, :], 2 * HB * blk * 2 * W + seg * segw,
                               [[2 * H * 2 * W, 128],
                                [128 * 2 * H * 2 * W, 2], [1, segw]])
                    src = _rap(oS[:, :], seg * segw,
                               [[OS_AL, 128], [2048, 2], [1, segw]])
                    nc.sync.dma_start(dst, src)
    return nc


def prep_features(features_f32):
    """(N, C, H, W) f32 -> list of (128, 8192) bf16 in fI layout
    fI[2w+par, k*C+c] = f[2k+par, w, c]."""
    ft = np.ascontiguousarray(features_f32.transpose(0, 2, 3, 1))  # (N, H, W, C)
    fi = ft.reshape(N, 32, 2, W, C).transpose(0, 3, 2, 1, 4).reshape(N, 128, 32 * C)
    fi = np.ascontiguousarray(fi).astype(ml_dtypes.bfloat16)
    return [fi[i] for i in range(N)]


def prep_masks(masks_f32):
    """(N, 25, 2H, 2W) f32 -> per-batch flat banded region buffers (bf16).

    Per region: T[n, blk, r, t, hl, a, jrel, b2] with r = 2*(w'-wb)+par,
    value = masks[5dy+dx, 16blk+2hl+a, 2(jlo+jrel)+b2] where
    dy = 2t+par-(hl&1), dx = w'-j+2, zero outside [0,5)."""
    n = masks_f32.shape[0]
    m = np.asarray(masks_f32, dtype=np.float32)
    flats = []
    for reg in REGIONS:
        wb, wn, jlo, jhi, rows, cw, cols = _reg_geom(reg)
        J = jhi - jlo
        T = np.zeros((n, NBLK, rows, 3, HB, 2, J, 2), np.float32)
        for t in range(3):
            for par in range(2):
                for hp in range(2):
                    dy = 2 * t + par - hp
                    if not 0 <= dy < 5:
                        continue
                    for dx in range(5):
                        jj = np.arange(max(jlo, wb - dx + 2),
                                       min(jhi, wb + wn - dx + 2))
                        if len(jj) == 0:
                            continue
                        ws = jj + dx - 2
                        rs = 2 * (ws - wb) + par
                        jrels = jj - jlo
                        plane = m[:, 5 * dy + dx]          # (n, 128, 128)
                        pr = plane.reshape(n, NBLK, 4, 2, 2, 128)[:, :, :, hp]
                        # (n, NBLK, 4, 2, 128): (blk, hl/2, a, ow)
                        owidx = (2 * jj)[:, None] + np.arange(2)[None, :]
                        sel = pr[..., owidx]               # (n, NBLK, 4, 2, J', 2)
                        sel = np.moveaxis(sel, 4, 0)       # (J', n, NBLK, 4, 2, 2)
                        T[:, :, rs, t, hp::2, :, jrels, :] = sel
        flats.append(T.reshape(n, -1))
    tope = np.concatenate(flats, axis=1).astype(ml_dtypes.bfloat16)
    return [tope[i] for i in range(n)]


_NC_CACHE = {}


def _get_nc(repeat=1):
    key = ("nc", repeat)
    if key not in _NC_CACHE:
        nc = bacc.Bacc()
        build_carafe(nc, out_dtype=mybir.dt.bfloat16, repeat=repeat)
        nc.compile()
        _NC_CACHE[key] = nc
    return _NC_CACHE[key]


def _in_maps(features, masks):
    fts = prep_features(np.asarray(features, dtype=np.float32))
    mbs = prep_masks(np.asarray(masks, dtype=np.float32))
    return [{"features": fts[i], "masks": mbs[i]} for i in range(N)]


def run_profiled(inputs):
    """Run with NTFF tracing; returns exec_time_ns (or None if unavailable)."""
    nc = _get_nc()
    res = run_bass_kernel_spmd(nc, _in_maps(inputs["features"], inputs["masks"]),
                               core_ids=list(range(N)), trace=True)
    return res.exec_time_ns


def bench(features, masks, reps=64, repeat=1):
    """Repeat-execute the compiled NEFF on all 8 cores; returns (per_iter_ns,
    first_call_s).  Upper bound on HW exec time (includes dispatch overhead)."""
    import time
    import jax
    from jax.sharding import Mesh, PartitionSpec
    from jax.experimental.shard_map import shard_map
    from concourse import bass2jax
    import concourse.mybir as mybir_

    nc = _get_nc(repeat)
    bass2jax.install_neuronx_cc_hook()
    in_maps = _in_maps(features, masks)

    in_names, out_names, out_avals, zero_outs = [], [], [], []
    for alloc in nc.m.functions[0].allocations:
        if not isinstance(mybir_.MemoryLocationSet, type) or not isinstance(alloc, mybir_.MemoryLocationSet):
            continue
        name = alloc.memorylocations[0].name
        pname = nc.partition_id_tensor.name if nc.partition_id_tensor else None
        if alloc.kind == "ExternalInput":
            if name != pname:
                in_names.append(name)
        elif alloc.kind == "ExternalOutput":
            out_names.append(name)
            shape = tuple(alloc.tensor_shape)
            dtype = mybir_.dt.np(alloc.dtype)
            out_avals.append(jax.core.ShapedArray(shape, dtype))
            zero_outs.append(np.zeros(shape, dtype))
    n_params = len(in_names)
    in_names = in_names + out_names
    if nc.partition_id_tensor is not None:
        in_names.append(nc.partition_id_tensor.name)

    def _body(*args):
        operands = list(args)
        if nc.partition_id_tensor is not None:
            operands.append(bass2jax.partition_id_tensor())
        outs = bass2jax._bass_exec_p.bind(
            *operands,
            out_avals=tuple(out_avals),
            in_names=tuple(in_names),
            out_names=tuple(out_names),
            lowering_input_output_aliases=(),
            sim_require_finite=True,
            sim_require_nnan=True,
            nc=nc,
        )
        return tuple(outs)

    devices = jax.devices()[:N]
    mesh = Mesh(np.asarray(devices), ("core",))
    nin = n_params + len(out_names)
    fn = jax.jit(
        shard_map(_body, mesh=mesh, in_specs=(PartitionSpec("core"),) * nin,
                  out_specs=(PartitionSpec("core"),) * len(out_names),
                  check_rep=False),
        keep_unused=True,
    )
    per_core = [[np.asarray(mp[k]) for k in in_names[:n_params]] for mp in in_maps]
    args = [np.concatenate([per_core[c][i] for c in range(N)], axis=0)
            for i in range(n_params)]
    args += [np.zeros((N * z.shape[0], *z.shape[1:]), z.dtype) for z in zero_outs]
    from jax.sharding import NamedSharding
    sh = NamedSharding(mesh, PartitionSpec("core"))
    args = [jax.device_put(a, sh) for a in args]
    t0 = time.time()
    outs = fn(*args)
    jax.block_until_ready(outs)
    first_s = time.time() - t0
    t0 = time.time()
    last = None
    for _ in range(reps):
        last = fn(*args)
    jax.block_until_ready(last)
    per_iter_ns = (time.time() - t0) / reps * 1e9
    return per_iter_ns, first_s


def kernel(features: np.ndarray, masks: np.ndarray) -> np.ndarray:
    nc = _get_nc()
    res = run_bass_kernel_spmd(nc, _in_maps(features, masks), core_ids=list(range(N)))
    return np.stack([np.asarray(res.results[i]["out"], dtype=np.float32)
                     for i in range(N)])


# revision 20
# speedup vs baseline: 1.0816x; 1.0816x over previous
"""CARAFE content-aware upsampling on 8 Trainium2 NeuronCores (v2).

Full inputs: features (8, 256, 64, 64) f32, masks (8, 25, 128, 128) f32.
Full output: (8, 256, 128, 128) f32.  Data-parallel: one batch per core.

Math per batch (kernel 5x5, group 1, scale 2, pad 2):
  out[c, 2h+a, 2j+b] = sum_{dy,dx} f[c, h+dy-2, j+dx-2] * masks[5dy+dx, 2h+a, 2j+b]

Device strategy (per input row h, per c-half): accumulate into PSUM via
bf16 matmuls whose contraction dim is (input-row-pair x w'-window):
  psum[c(128), (a,ow)] += lhsT[p=(w',par), c].T @ T[p, (a,ow)]
lhsT = features in interleaved layout fI[p = 2w+par, k*C+c] = f[2k+par, w, c],
so a row-pair restricted to a w'-window is a contiguous partition range.
T = host-prebuilt banded mask (Toeplitz) tiles.  The w' windows are limited
by the PE base-partition rule (base in {0,32,64}, base-32 spans <= 32):

  R0: w' [0,20)  base 0,  covers j [0,18)   (ow [0,36))
  R1: w' [16,32) base 32, covers j [18,30)  (ow [36,60))
  RP: w' [0,36)  base 0,  covers j [30,34)  (ow [60,68))
  R2: w' [32,64) base 64, covers j [34,64)  (ow [68,128))

Each (h, region) is a <=3-link PSUM chain over row pairs P_{m-1}, P_m,
P_{m+1} (m = h>>1); slots whose dy falls outside [0,5) carry zeros in T.
Matmul out APs are 3-dim strided so PSUM ends up row-major (a, ow) and a
single [128,256] copy per (h, half) moves it to SBUF as bf16.

Banded tiles carry ~4.9 MiB instead of 10 MiB (baseline), features load
once (2 MiB instead of 6), output is 8 MiB bf16: ~15 MiB HBM traffic/core.
"""

import os
import sys

_WARM = int(os.environ.get("K_WARM", "16"))  # PE warm-up dummy matmul count

if "/opt/trn_rl_repo" not in sys.path:
    sys.path.append("/opt/trn_rl_repo")

from contextlib import ExitStack

import numpy as np
import ml_dtypes

import concourse.bass as bass
import concourse.bacc as bacc
import concourse.mybir as mybir
import concourse.tile as tile
from concourse.ap import AP
from concourse.bass_utils import run_bass_kernel_spmd

N = 8
C = 256
H = 64
W = 64
HB = 8                       # input rows per block
NBLK = H // HB
FI_T = 2048                  # free elems per fI sub-tile (8 row-pairs x C)
OS_F = 2 * HB * 2 * 2 * W    # 4096 outS cols per block (half, hl, a, ow)
OS_AL = OS_F + 64

# (wb, wn, jlo, jhi): w'-window [wb, wb+wn), output-col range j [jlo, jhi)
REGIONS = [
    (0, 20, 0, 18),
    (16, 16, 18, 30),
    (0, 36, 30, 34),
    (32, 32, 34, 64),
]


def _reg_geom(reg):
    wb, wn, jlo, jhi = reg
    rows = 2 * wn
    cw = 2 * (jhi - jlo)          # output cols in chunk
    cols = 3 * HB * 2 * (jhi - jlo) * 2   # (t, hl, a, jrel, b2)
    return wb, wn, jlo, jhi, rows, cw, cols


def _rap(tile_ap, off, dims):
    return AP(tile_ap.tensor, tile_ap.offset + off, dims)


def build_carafe(nc, out_dtype=mybir.dt.bfloat16, repeat=1):
    total_mask = sum(NBLK * _reg_geom(r)[4] * _reg_geom(r)[6] for r in REGIONS)
    feat = nc.declare_dram_parameter("features", (128, 4 * FI_T), mybir.dt.bfloat16,
                                     isOutput=False)
    tope = nc.declare_dram_parameter("masks", (total_mask,), mybir.dt.bfloat16,
                                     isOutput=False)
    out = nc.declare_dram_parameter("out", (C, 2 * H, 2 * W), out_dtype, isOutput=True)

    ctx = ExitStack()
    with ctx:
        tc = ctx.enter_context(tile.TileContext(nc))
        pool = ctx.enter_context(tc.tile_pool(name="main", bufs=1))
        ppool = ctx.enter_context(tc.tile_pool(name="psum", bufs=1, space="PSUM"))

        # features: fI0a holds pairs 0-4 (all block 0 needs), fI0b pairs 5-7,
        # then three 8-pair tiles.  Pair k lives in tile _fi_tile(k) at column
        # _fi_off(k).
        fI0a = pool.tile([128, 5 * C], mybir.dt.bfloat16, tag="fI0a", name="fI0a")
        fI0b = pool.tile([128, 3 * C], mybir.dt.bfloat16, tag="fI0b", name="fI0b")
        fIq = [pool.tile([128, FI_T], mybir.dt.bfloat16, tag=f"fI{q}", name=f"fI{q}")
               for q in (1, 2, 3)]

        def _fi(k):
            if k < 5:
                return fI0a, k * C, 5 * C
            if k < 8:
                return fI0b, (k - 5) * C, 3 * C
            return fIq[(k >> 3) - 1], (k & 7) * C, FI_T

        def load_fi(q):
            nc.sync.dma_start(
                _rap(fIq[q - 1][:, :], 0, [[FI_T, 128], [1, FI_T]]),
                _rap(feat[:, :], q * FI_T, [[4 * FI_T, 128], [1, FI_T]]))

        # banded mask tiles: ring of 2 per region; rows live at partitions
        # [2*wb, 2*wb+rows)
        tT = []
        for ri, reg in enumerate(REGIONS):
            wb, wn, jlo, jhi, rows, cw, cols = _reg_geom(reg)
            tT.append([pool.tile([2 * wb + rows, cols], mybir.dt.bfloat16,
                                 tag=f"t{ri}_{i}", name=f"t{ri}_{i}")
                       for i in range(2)])

        outS = [pool.tile([128, OS_AL], out_dtype, tag=f"outS_{i}", name=f"outS_{i}")
                for i in range(2)]
        psum = [ppool.tile([128, 512], mybir.dt.float32, tag=f"ps_{i}", name=f"ps_{i}")
                for i in range(8)]

        reg_base = []
        acc = 0
        for reg in REGIONS:
            reg_base.append(acc)
            acc += NBLK * _reg_geom(reg)[4] * _reg_geom(reg)[6]

        copy_engines = [nc.vector.tensor_copy, nc.scalar.copy]

        def load_masks(it, order=(0, 1, 2, 3)):
            blk = it % NBLK
            ring = it % 2
            for ri in order:
                reg = REGIONS[ri]
                wb, wn, jlo, jhi, rows, cw, cols = _reg_geom(reg)
                t = tT[ri][ring]
                eng = nc.scalar if ri % 2 == 0 else nc.sync
                eng.dma_start(
                    _rap(t[:, :], 2 * wb * cols, [[cols, rows], [1, cols]]),
                    _rap(tope[:], reg_base[ri] + blk * rows * cols,
                         [[cols, rows], [1, cols]]))

        niter = NBLK * repeat
        if _WARM:
            # ramp the PE p-state during the DMA-bound startup: dummy matmuls
            # on a zeroed scratch tile into psum[7] (overwritten by its first
            # real chain)
            zt = pool.tile([128, 128], mybir.dt.bfloat16, tag="zt", name="zt")
            nc.vector.memset(zt[:, :], 0.0)
            zl = _rap(zt[:, :], 0, [[128, 128], [1, 128]])
            for _ in range(_WARM):
                nc.tensor.matmul(psum[7][:, 0:128], zl, zl, start=True, stop=True)
        # startup order: fI0a and R0's tile gate the first matmuls -- issue
        # them at the head of their queues, fI0b behind the block-0 masks
        nc.sync.dma_start(
            _rap(fI0a[:, :], 0, [[5 * C, 128], [1, 5 * C]]),
            _rap(feat[:, :], 0, [[4 * FI_T, 128], [1, 5 * C]]))
        load_masks(0)
        nc.scalar.dma_start(
            _rap(fI0b[:, :], 0, [[3 * C, 128], [1, 3 * C]]),
            _rap(feat[:, :], 5 * C, [[4 * FI_T, 128], [1, 3 * C]]))
        for it in range(niter):
            blk = it % NBLK
            ring = it % 2
            # prefetch next block's banded tiles before this block's compute
            if it + 1 < niter:
                load_masks(it + 1)
            if it == 0:
                for q in (1, 2, 3):
                    load_fi(q)
            oS = outS[ring]

            def chain(hl, half, ri):
                h = HB * blk + hl
                m = h >> 1
                links = [t_ for t_ in range(3) if 0 <= m - 1 + t_ < 32]
                wb, wn, jlo, jhi, rows, cw, cols = _reg_geom(REGIONS[ri])
                t = tT[ri][ring]
                ps = psum[(2 * h + half) % 8]
                out_ap = _rap(ps[:, :], 2 * jlo, [[512, 128], [128, 2], [1, cw]])
                for i, tt in enumerate(links):
                    k = m - 1 + tt
                    ft, foff, fpitch = _fi(k)
                    lhs = _rap(ft[:, :],
                               2 * wb * fpitch + foff + half * 128,
                               [[fpitch, rows], [1, 128]])
                    rhs = _rap(t[:, :],
                               2 * wb * cols + tt * HB * 2 * cw + hl * 2 * cw,
                               [[cols, rows], [1, 2 * cw]])
                    nc.tensor.matmul(out_ap, lhs, rhs,
                                     start=(i == 0), stop=(i == len(links) - 1))

            def copy_out(hl, half):
                h = HB * blk + hl
                ps = psum[(2 * h + half) % 8]
                cp = copy_engines[(2 * h + half) % 2]
                cp(oS[:, half * 2048 + hl * 256:half * 2048 + (hl + 1) * 256],
                   ps[:, 0:256])

            for hl in range(HB):
                if it == 0 and hl % 4 == 0:
                    # block 0: region-major over the half-block so PE starts
                    # as soon as the first region tile lands
                    for ri in range(len(REGIONS)):
                        for hl2 in range(hl, hl + 4):
                            for half in (0, 1):
                                chain(hl2, half, ri)
                if it == 0:
                    for half in (0, 1):
                        copy_out(hl, half)
                else:
                    for half in (0, 1):
                        for ri in range(len(REGIONS)):
                            chain(hl, half, ri)
                        copy_out(hl, half)
                fine = it == niter - 1
                if (hl % 2 == 1) if fine else (hl in (3, 7)):
                    segw = 512 if fine else 1024
                    seg = hl // 2 if fine else hl // 4
                    dst = _rap(out[:, :, :], 2 * HB * blk * 2 * W + seg * segw,
                               [[2 * H * 2 * W, 128],
                                [128 * 2 * H * 2 * W, 2], [1, segw]])
                    src = _rap(oS[:, :], seg * segw,
                               [[OS_AL, 128], [2048, 2], [1, segw]])
                    nc.sync.dma_start(dst, src)
    return nc


def prep_features(features_f32):
    """(N, C, H, W) f32 -> list of (128, 8192) bf16 in fI layout
    fI[2w+par, k*C+c] = f[2k+par, w, c]."""
    ft = np.ascontiguousarray(features_f32.transpose(0, 2, 3, 1))  # (N, H, W, C)
    fi = ft.reshape(N, 32, 2, W, C).transpose(0, 3, 2, 1, 4).reshape(N, 128, 32 * C)
    fi = np.ascontiguousarray(fi).astype(ml_dtypes.bfloat16)
    return [fi[i] for i in range(N)]


def prep_masks(masks_f32):
    """(N, 25, 2H, 2W) f32 -> per-batch flat banded region buffers (bf16).

    Per region: T[n, blk, r, t, hl, a, jrel, b2] with r = 2*(w'-wb)+par,
    value = masks[5dy+dx, 16blk+2hl+a, 2(jlo+jrel)+b2] where
    dy = 2t+par-(hl&1), dx = w'-j+2, zero outside [0,5)."""
    n = masks_f32.shape[0]
    m = np.asarray(masks_f32, dtype=np.float32)
    flats = []
    for reg in REGIONS:
        wb, wn, jlo, jhi, rows, cw, cols = _reg_geom(reg)
        J = jhi - jlo
        T = np.zeros((n, NBLK, rows, 3, HB, 2, J, 2), np.float32)
        for t in range(3):
            for par in range(2):
                for hp in range(2):
                    dy = 2 * t + par - hp
                    if not 0 <= dy < 5:
                        continue
                    for dx in range(5):
                        jj = np.arange(max(jlo, wb - dx + 2),
                                       min(jhi, wb + wn - dx + 2))
                        if len(jj) == 0:
                            continue
                        ws = jj + dx - 2
                        rs = 2 * (ws - wb) + par
                        jrels = jj - jlo
                        plane = m[:, 5 * dy + dx]          # (n, 128, 128)
                        pr = plane.reshape(n, NBLK, 4, 2, 2, 128)[:, :, :, hp]
                        # (n, NBLK, 4, 2, 128): (blk, hl/2, a, ow)
                        owidx = (2 * jj)[:, None] + np.arange(2)[None, :]
                        sel = pr[..., owidx]               # (n, NBLK, 4, 2, J', 2)
                        sel = np.moveaxis(sel, 4, 0)       # (J', n, NBLK, 4, 2, 2)
                        T[:, :, rs, t, hp::2, :, jrels, :] = sel
        flats.append(T.reshape(n, -1))
    tope = np.concatenate(flats, axis=1).astype(ml_dtypes.bfloat16)
    return [tope[i] for i in range(n)]


_NC_CACHE = {}


def _get_nc(repeat=1):
    key = ("nc", repeat)
    if key not in _NC_CACHE:
        nc = bacc.Bacc()
        build_carafe(nc, out_dtype=mybir.dt.bfloat16, repeat=repeat)
        nc.compile()
        _NC_CACHE[key] = nc
    return _NC_CACHE[key]


def _in_maps(features, masks):
    fts = prep_features(np.asarray(features, dtype=np.float32))
    mbs = prep_masks(np.asarray(masks, dtype=np.float32))
    return [{"features": fts[i], "masks": mbs[i]} for i in range(N)]


def run_profiled(inputs):
    """Run with NTFF tracing; returns exec_time_ns (or None if unavailable)."""
    nc = _get_nc()
    res = run_bass_kernel_spmd(nc, _in_maps(inputs["features"], inputs["masks"]),
                               core_ids=list(range(N)), trace=True)
    return res.exec_time_ns


def bench(features, masks, reps=64, repeat=1):
    """Repeat-execute the compiled NEFF on all 8 cores; returns (per_iter_ns,
    first_call_s).  Upper bound on HW exec time (includes dispatch overhead)."""
    import time
    import jax
    from jax.sharding import Mesh, PartitionSpec
    from jax.experimental.shard_map import shard_map
    from concourse import bass2jax
    import concourse.mybir as mybir_

    nc = _get_nc(repeat)
    bass2jax.install_neuronx_cc_hook()
    in_maps = _in_maps(features, masks)

    in_names, out_names, out_avals, zero_outs = [], [], [], []
    for alloc in nc.m.functions[0].allocations:
        if not isinstance(mybir_.MemoryLocationSet, type) or not isinstance(alloc, mybir_.MemoryLocationSet):
            continue
        name = alloc.memorylocations[0].name
        pname = nc.partition_id_tensor.name if nc.partition_id_tensor else None
        if alloc.kind == "ExternalInput":
            if name != pname:
                in_names.append(name)
        elif alloc.kind == "ExternalOutput":
            out_names.append(name)
            shape = tuple(alloc.tensor_shape)
            dtype = mybir_.dt.np(alloc.dtype)
            out_avals.append(jax.core.ShapedArray(shape, dtype))
            zero_outs.append(np.zeros(shape, dtype))
    n_params = len(in_names)
    in_names = in_names + out_names
    if nc.partition_id_tensor is not None:
        in_names.append(nc.partition_id_tensor.name)

    def _body(*args):
        operands = list(args)
        if nc.partition_id_tensor is not None:
            operands.append(bass2jax.partition_id_tensor())
        outs = bass2jax._bass_exec_p.bind(
            *operands,
            out_avals=tuple(out_avals),
            in_names=tuple(in_names),
            out_names=tuple(out_names),
            lowering_input_output_aliases=(),
            sim_require_finite=True,
            sim_require_nnan=True,
            nc=nc,
        )
        return tuple(outs)

    devices = jax.devices()[:N]
    mesh = Mesh(np.asarray(devices), ("core",))
    nin = n_params + len(out_names)
    fn = jax.jit(
        shard_map(_body, mesh=mesh, in_specs=(PartitionSpec("core"),) * nin,
                  out_specs=(PartitionSpec("core"),) * len(out_names),
                  check_rep=False),
        keep_unused=True,
    )
    per_core = [[np.asarray(mp[k]) for k in in_names[:n_params]] for mp in in_maps]
    args = [np.concatenate([per_core[c][i] for c in range(N)], axis=0)
            for i in range(n_params)]
    args += [np.zeros((N * z.shape[0], *z.shape[1:]), z.dtype) for z in zero_outs]
    from jax.sharding import NamedSharding
    sh = NamedSharding(mesh, PartitionSpec("core"))
    args = [jax.device_put(a, sh) for a in args]
    t0 = time.time()
    outs = fn(*args)
    jax.block_until_ready(outs)
    first_s = time.time() - t0
    t0 = time.time()
    last = None
    for _ in range(reps):
        last = fn(*args)
    jax.block_until_ready(last)
    per_iter_ns = (time.time() - t0) / reps * 1e9
    return per_iter_ns, first_s


def kernel(features: np.ndarray, masks: np.ndarray) -> np.ndarray:
    nc = _get_nc()
    res = run_bass_kernel_spmd(nc, _in_maps(features, masks), core_ids=list(range(N)))
    return np.stack([np.asarray(res.results[i]["out"], dtype=np.float32)
                     for i in range(N)])
